# revision 7
# baseline (speedup 1.0000x reference)
"""2x2/stride-2 max-pool (NCHW, padding=0) on Trainium2, data-parallel over 8 cores.

Problem: x (32, 96, 224, 224) fp32 -> out (32, 96, 112, 112) fp32.

Sharding: pure data parallel on the batch dim — core i handles x[4i:4i+4].
Per core the (4, 96, 224, 224) shard is viewed as 43008 row-pairs of 448
contiguous floats ((n,c,h-pair) x (2 rows * 224 cols)).  Each chunk loads a
fully contiguous [128 partitions x Mc row-pairs] block, reduces it with two
elementwise-max stages on DVE/ACT (vertical rows in place, then horizontal
column pairs), and stores a fully contiguous [128 x Mc*112] block.  Main
chunks use Mc=21 (4.8 MiB loads, ~97% of DMA fabric rate); the final chunk
descends (12/6/2/1) so the end-of-kernel load->max->max->store chain is short.
"""

import numpy as np

N_CORES = 8
PAIRS = 43008               # row-pairs per core: 4*96*224/2
M_MAIN = 21                 # row-pairs per partition per main chunk
N_MAIN = 15                 # main chunks
TAIL = [12, 6, 2, 1]        # descending tail chunk sizes (sum 21)
IN_SHAPE = (32, 96, 224, 224)
H_OUT = 112

assert N_MAIN * M_MAIN + sum(TAIL) == PAIRS // 128

_cache = {}


def _build():
    import concourse.bass as bass  # noqa: F401
    import concourse.tile as tile
    from concourse import bacc, mybir

    nc = bacc.Bacc("TRN2", target_bir_lowering=False, debug=False)
    x = nc.dram_tensor("x", [PAIRS, 448], mybir.dt.float32, kind="ExternalInput")
    o = nc.dram_tensor("o", [PAIRS, 112], mybir.dt.float32, kind="ExternalOutput")
    xap, oap = x.ap(), o.ap()

    chunks = []
    base = 0
    for mc in [M_MAIN] * N_MAIN + TAIL:
        chunks.append((base, mc))
        base += 128 * mc

    with tile.TileContext(nc) as tc:
        with (
            tc.tile_pool(name="inp", bufs=4) as pin,
            tc.tile_pool(name="outp", bufs=3) as po,
        ):
            for i, (base, mc) in enumerate(chunks):
                src = xap[base : base + 128 * mc].rearrange("(p m) w -> p (m w)", p=128)
                dst = oap[base : base + 128 * mc].rearrange("(p m) w -> p (m w)", p=128)
                tin = pin.tile([128, mc, 2, 112, 2], mybir.dt.float32)
                # loads and stores ride opposite HWDGE rings (SP vs ACT),
                # alternating per chunk, so read/write packets interleave and
                # a store never queues behind the next load in one ring FIFO
                ld, st = (nc.sync, nc.scalar) if i % 2 == 0 else (nc.scalar, nc.sync)
                ld.dma_start(out=tin[:], in_=src)
                # vertical max of the two pooled rows, in place into row 0
                nc.any.tensor_max(tin[:, :, 0], tin[:, :, 0], tin[:, :, 1])
                to = po.tile([128, mc, 112], mybir.dt.float32)
                # horizontal max of adjacent column pairs
                nc.any.tensor_max(to[:], tin[:, :, 0, :, 0], tin[:, :, 0, :, 1])
                st.dma_start(out=dst, in_=to[:])
    nc.compile()
    return nc


def get_nc():
    if "nc" not in _cache:
        _cache["nc"] = _build()
    return _cache["nc"]


def shard(x: np.ndarray, c: int) -> dict:
    per = IN_SHAPE[0] // N_CORES
    return {
        "x": np.ascontiguousarray(x[c * per : (c + 1) * per]).reshape(PAIRS, 448)
    }


def unshard(outs: list) -> np.ndarray:
    per = IN_SHAPE[0] // N_CORES
    return np.concatenate(
        [o.reshape(per, IN_SHAPE[1], H_OUT, H_OUT) for o in outs], axis=0
    )


def kernel(x: np.ndarray) -> np.ndarray:
    from concourse.bass_utils import run_bass_kernel_spmd

    assert x.shape == IN_SHAPE and x.dtype == np.float32, (x.shape, x.dtype)
    nc = get_nc()
    in_maps = [shard(x, c) for c in range(N_CORES)]
    res = run_bass_kernel_spmd(nc, in_maps, list(range(N_CORES)))
    return unshard([res.results[c]["o"] for c in range(N_CORES)])


# revision 8
# speedup vs baseline: 1.1123x; 1.1123x over previous
"""2x2/stride-2 max-pool (NCHW, padding=0) on Trainium2, data-parallel over 8 cores.

Problem: x (32, 96, 224, 224) fp32 -> out (32, 96, 112, 112) fp32.

Sharding: pure data parallel on the batch dim — core i handles x[4i:4i+4].
Per core the (4, 96, 224, 224) shard is viewed as 43008 row-pairs of 448
contiguous floats ((n,c,h-pair) x (2 rows * 224 cols)).  Each chunk loads a
fully contiguous [128 partitions x Mc row-pairs] block, reduces it with two
elementwise-max stages on DVE/ACT (vertical rows in place, then horizontal
column pairs), and stores a fully contiguous [128 x Mc*112] block.  Main
chunks use Mc=21 (4.8 MiB loads, ~97% of DMA fabric rate); the final chunk
descends (12/6/2/1) so the end-of-kernel load->max->max->store chain is short.
"""

import numpy as np

N_CORES = 8
PAIRS = 43008               # row-pairs per core: 4*96*224/2
M_MAIN = 21                 # row-pairs per partition per main chunk
N_MAIN = 15                 # main chunks
TAIL = [12, 6, 2, 1]        # descending tail chunk sizes (sum 21)
IN_SHAPE = (32, 96, 224, 224)
H_OUT = 112

assert N_MAIN * M_MAIN + sum(TAIL) == PAIRS // 128

_cache = {}


def _build():
    import concourse.bass as bass  # noqa: F401
    import concourse.tile as tile
    from concourse import bacc, mybir

    nc = bacc.Bacc("TRN2", target_bir_lowering=False, debug=False)
    x = nc.dram_tensor("x", [PAIRS, 448], mybir.dt.float32, kind="ExternalInput")
    o = nc.dram_tensor("o", [PAIRS, 112], mybir.dt.float32, kind="ExternalOutput")
    xap, oap = x.ap(), o.ap()

    chunks = []
    base = 0
    for mc in [M_MAIN] * N_MAIN + TAIL:
        chunks.append((base, mc))
        base += 128 * mc

    with tile.TileContext(nc) as tc:
        with (
            tc.tile_pool(name="inp", bufs=4) as pin,
            tc.tile_pool(name="outp", bufs=3) as po,
        ):
            for base, mc in chunks:
                src = xap[base : base + 128 * mc].rearrange("(p m) w -> p (m w)", p=128)
                dst = oap[base : base + 128 * mc].rearrange("(p m) w -> p (m w)", p=128)
                tin = pin.tile([128, mc, 2, 112, 2], mybir.dt.float32)
                nc.sync.dma_start(out=tin[:], in_=src)
                # vertical max of the two pooled rows, in place into row 0
                nc.any.tensor_max(tin[:, :, 0], tin[:, :, 0], tin[:, :, 1])
                to = po.tile([128, mc, 112], mybir.dt.float32)
                # horizontal max of adjacent column pairs
                nc.any.tensor_max(to[:], tin[:, :, 0, :, 0], tin[:, :, 0, :, 1])
                # stores ride the ACT HWDGE ring: keeping each ring dedicated
                # to one direction beats alternating (measured) — a store
                # never queues behind the next load in the SP ring's FIFO
                nc.scalar.dma_start(out=dst, in_=to[:])
    nc.compile()
    return nc


def get_nc():
    if "nc" not in _cache:
        _cache["nc"] = _build()
    return _cache["nc"]


def shard(x: np.ndarray, c: int) -> dict:
    per = IN_SHAPE[0] // N_CORES
    return {
        "x": np.ascontiguousarray(x[c * per : (c + 1) * per]).reshape(PAIRS, 448)
    }


def unshard(outs: list) -> np.ndarray:
    per = IN_SHAPE[0] // N_CORES
    return np.concatenate(
        [o.reshape(per, IN_SHAPE[1], H_OUT, H_OUT) for o in outs], axis=0
    )


def kernel(x: np.ndarray) -> np.ndarray:
    from concourse.bass_utils import run_bass_kernel_spmd

    assert x.shape == IN_SHAPE and x.dtype == np.float32, (x.shape, x.dtype)
    nc = get_nc()
    in_maps = [shard(x, c) for c in range(N_CORES)]
    res = run_bass_kernel_spmd(nc, in_maps, list(range(N_CORES)))
    return unshard([res.results[c]["o"] for c in range(N_CORES)])
